# revision 24
# baseline (speedup 1.0000x reference)
"""Trainium2 Bass kernel for nn_BasicClassifier (spiking conv classifier).

Sharding: pure data parallelism — batch 256 is split 32 samples per core
across 8 NeuronCores; params are replicated (tiny).

Per-core design. The T=1000 LIF scan is sequential, so per-step cost on the
pacing engine (DVE) decides everything.  The key trick: ONE custom-DVE op
per 16-tick block computes all 16 sequential LIF steps, exploiting the
DVE's in-instruction write->read visibility at a 128-element lag (measured
on HW: reads of an address written by the same instruction 128 cycles
earlier return the new value; at <=96 cycles they race).

State is t-major (tick-slot-major) with seed slots:
  ring tile (fp32 SBUF, 2 tiles of TBLK=2 blocks each): 33 slots of
  [128 cols]; slot 0 = seed (m one tick before the tile), slot 1+k = m
  after tick k.  Within a slot: cols g*32+s, groups 0..2 = layer-1
  membrane (feature f = g*128+p at partition p), group 3 = layer-2
  membrane [35 x 32], lagged SKEW=48 ticks.

  Per block the LIF op is  m' = (m*0.9 + c) - (m > 1)  over 2048
  contiguous elements (16 slots x 128) with in0 = out SHIFTED ONE SLOT
  (128 elements = 128 DVE cycles): 16 per-tick instructions (~320ns
  each) collapse into one ~2.3us op (measured: the op runs 1 elem/cycle
  with the SBUF fp32 in1 — no read-port penalty at this shape).  A tiny
  DVE copy seeds slot 0 of the next ring tile every TBLK blocks.

Per 16-tick block (all off the tick-critical path; LAG=3 blocks of
layer-2 lag gives every pipeline stage a full block-period of slack):
  - PSUM C tile [128, 4*512] (bank-contiguous, g-major — the PE cannot
    write strided cross-bank outputs): banks 0-2 = conv1d hoisted into
    [31 -> 384] GEMMs (bf16 hi/lo K-stacked: xh@Wh + xl@Wh + xh@Wl in
    one K=93 matmul per group; ones row folds conv_b), bank 3 = fc bias
    prefill (K=2 bf16 hi/lo matmul, exact) + 3 fp16 fc matmuls from the
    spikes of block b-LAG.
  - staging: 4 ACT copies permute the PSUM tile into the t-major fp32
    SBUF ctile the LIF op consumes ([128,512] bank g -> strided
    [p][slot:128][32 @ g*32]).  Exact (fp32 -> fp32).
  - spikes: 3 GPSIMD tensor_scalar ops (strided [128,16,32] in,
    group-contiguous out): s = (m1 > 1) in {0,1} fp16; fc weights are
    plain fc_w fp16 (no sigma folding needed).
  - mem2 history: DMA of ring slot cols 96:128 to fp32 [35, 32*T] DRAM
    (host sums over time).
x is streamed in 16-tick chunk DMAs on the sync queue (chunks small
enough that the per-block hist DMAs are never head-of-line blocked for
long), prefetched ~2 windows ahead.
"""

import os
import sys

for _p in ("/opt/trn_rl_repo", "/opt/pypackages"):
    if _p not in sys.path:
        sys.path.insert(0, _p)

import numpy as np

import concourse.bacc as bacc
import concourse.mybir as mybir
import concourse.tile as tile
import concourse.dve_ops as dve_ops
from concourse.dve_spec import Spec, Src0, Src1, C0, C1, lower
from concourse.dve_uop import DveOpSpec
from concourse.bass_utils import run_bass_kernel_spmd

F32 = mybir.dt.float32
F16 = mybir.dt.float16
BF16 = mybir.dt.bfloat16
ALU = mybir.AluOpType
AF = mybir.ActivationFunctionType
AX = mybir.AxisListType

N_CORES = 8
B_FULL, T_FULL, L_IN = 256, 1000, 30
BC = B_FULL // N_CORES      # 32 samples per core
CH, LO = 16, 24
F = CH * LO                 # 384 features
G = 3                       # feature groups of 128
J = 35                      # fc outputs
KX = L_IN + 1               # conv contraction rows (30 taps + ones row)
BLK = 16                    # ticks per block (N = 16*32 = 512 = 1 PSUM bank)
LAG = 3                     # layer-2 lag in blocks
SKEW = LAG * BLK            # c2_t consumed at DVE tick t+SKEW
WIN = 160                   # ticks per x-window (chunk-DMAed per block)
TBLK = 2                    # ring tile = TBLK blocks
BETA, THR = 0.9, 1.0
SIGN_ON_ACT = False         # fallback: ACT Sign + sigma/halved-weight trick

TRACE = bool(int(os.environ.get("KERNEL_TRACE", "0")))
LAST_RESULTS = None

_LIF_OP = None


def _get_lif_op():
    """Register the fused LIF-step op in the custom-DVE table (idempotent)."""
    global _LIF_OP
    if _LIF_OP is not None:
        return _LIF_OP
    name = "LIF_STEP_ANT59"
    for op in dve_ops.OPS:
        if op.name == name:
            _LIF_OP = op
            return op
    spec = Spec(
        body=(Src0 * C0 + Src1) - (Src0 > C1),
        reference=lambda in0, in1, s0, s1, imm2: (
            (in0.astype(np.float32) * np.float32(s0)
             + in1.reshape(in0.shape))
            - (in0 > s1).astype(np.float32)
        ).astype(np.float32),
    )
    row = dve_ops._CUSTOM_DVE_ROW_BASE + len(dve_ops.OPS)
    assert row < 0x20
    dve_ops._SUB_OPCODE_FOR_NAME[name] = row
    shas = {}
    for ver in ("v3", "v4"):
        tmp = DveOpSpec(name=name, opcode=row, uops=lower(spec, ver=ver), rd1_en=True)
        shas[ver] = tmp.sha(ver)
    op = dve_ops.DveOp(name, spec, subdim=False, uops_sha=shas)
    dve_ops.OPS.append(op)
    dve_ops.CUSTOM_DVE_SPECS[name] = spec
    _LIF_OP = op
    return op


def _build_nc(T):
    """Build the per-core Bass program (SPMD: same program on every core)."""
    lif = _get_lif_op()
    ticks = T + SKEW                       # DVE ticks 0..T+SKEW-1
    nblk = -(-ticks // BLK)
    pad_ticks = nblk * BLK
    windows = -(-pad_ticks // WIN)
    xt_cols = windows * WIN * BC
    fcblk = -(-T // BLK)                   # blocks that need spikes/fc
    NB = BLK * BC                          # 512: one PSUM bank of f32
    SLOTS = TBLK * BLK + 1                 # ring slots (33), 128 cols each

    nc = bacc.Bacc("TRN2", target_bir_lowering=False)

    KS = 3 * KX                            # stacked conv K: [xh; xl; xh]
    xts_d = nc.dram_tensor("xts", [KS, xt_cols], BF16, kind="ExternalInput")
    wes_d = nc.dram_tensor("wes", [KS, F], BF16, kind="ExternalInput")
    fcw_d = nc.dram_tensor("fcw", [128, G * J], F16, kind="ExternalInput")
    brs_d = nc.dram_tensor("brs", [2, 128], BF16, kind="ExternalInput")
    ones_d = nc.dram_tensor("ones", [2, NB], BF16, kind="ExternalInput")
    hist_d = nc.dram_tensor("hist", [J, BC * T], F32, kind="ExternalOutput")

    with tile.TileContext(nc) as tc:
        with (
            tc.tile_pool(name="konst", bufs=1) as kp,
            tc.tile_pool(name="ring", bufs=1) as rp,
            tc.tile_pool(name="sig", bufs=2) as sgp,
            tc.tile_pool(name="ct", bufs=2) as ctp,
            tc.tile_pool(name="xwin", bufs=3) as xp,
            tc.tile_pool(name="cpsum", bufs=2, space="PSUM") as cp,
        ):
            # constants -> SBUF
            wes = kp.tile([KS, F], BF16, tag="wes")
            fcw = kp.tile([128, G * J], F16, tag="fcw")
            brs = kp.tile([2, 128], BF16, tag="brs")
            ones = kp.tile([2, NB], BF16, tag="ones")
            negthr = kp.tile([128, 1], F32, tag="negthr")
            nc.vector.memset(negthr[:], -THR)
            for sb, dr in ((wes, wes_d), (fcw, fcw_d),
                           (brs, brs_d), (ones, ones_d)):
                nc.sync.dma_start(sb[:], dr[:])

            # state ring: 2 tiles of TBLK blocks (t-major, 33 slots x 128)
            ringA = rp.tile([128, SLOTS * 128], F32, tag="ringA")
            ringB = rp.tile([128, SLOTS * 128], F32, tag="ringB")
            nc.vector.memset(ringA[:], 0.0)
            nc.vector.memset(ringB[:], 0.0)
            rings = (ringA, ringB)

            xts = {}      # window idx -> xt sbuf tile
            chs = {}      # block idx -> PSUM C tile [128, 4*512] g-major
            cts = {}      # block idx -> SBUF C tile [128, 2048] t-major
            sgs = {}      # block idx -> spike tile

            def rtile(b):
                """(ring tile, block-within-tile h) for block b."""
                return rings[(b // TBLK) % 2], b % TBLK

            def load_window(w):
                """x window as per-block chunk DMAs on the sync queue;
                small chunks never head-of-line-block a hist DMA for more
                than ~1.5us on the DMA engine."""
                if w >= windows or w in xts:
                    return
                ts = xp.tile([KS, WIN * BC], BF16, tag="xws")
                for c in range(WIN // BLK):
                    nc.sync.dma_start(
                        ts[:, c * NB:(c + 1) * NB],
                        xts_d[:, w * WIN * BC + c * NB:
                                 w * WIN * BC + (c + 1) * NB],
                    )
                xts[w] = ts

            def psum_bias(b):
                """Allocate block b's PSUM C tile; prime bank 3 with the fc
                bias (zeros when block b-LAG has no spikes)."""
                if b >= nblk or b in chs:
                    return
                ch = cp.tile([128, 4 * NB], F32, tag="ch")
                chs[b] = ch
                if b >= LAG and b - LAG < fcblk:
                    nc.tensor.matmul(
                        out=ch[:, G * NB:4 * NB],
                        lhsT=brs[:, :], rhs=ones[:, :],
                        start=True, stop=False,
                        skip_group_check=True,
                    )
                else:
                    nc.vector.memset(ch[:, G * NB:4 * NB], 0.0)

            def psum_conv(b):
                """conv-fill banks 0-2 of block b's PSUM C tile."""
                if b >= nblk:
                    return
                ch = chs[b]
                w = (b * BLK) // WIN
                base = (b * BLK - w * WIN) * BC
                for g in range(G):
                    nc.tensor.matmul(
                        out=ch[:, g * NB:(g + 1) * NB],
                        lhsT=wes[:, g * 128:(g + 1) * 128],
                        rhs=xts[w][:, base:base + NB],
                        start=True, stop=True,
                    )

            def spikes(b):
                """s = (m1 > 1) in {0,1} fp16 — 3 GPSIMD ops (strided in,
                group-contiguous out) so the otherwise idle GPSIMD carries
                them and ACT keeps headroom for the staging copies."""
                if b < 0 or b >= fcblk:
                    return
                ring, h = rtile(b)
                rv = ring[:].rearrange("p (t c) -> p t c", c=128)
                s0 = 1 + h * BLK
                sg = sgp.tile([128, G * NB], F16, tag="sg")
                sgs[b] = sg
                for g in range(G):
                    if SIGN_ON_ACT:
                        nc.scalar.activation(
                            out=sg[:, g * NB:(g + 1) * NB],
                            in_=rv[:, s0:s0 + BLK, g * BC:(g + 1) * BC],
                            func=AF.Sign, bias=negthr[:],
                        )
                    else:
                        nc.gpsimd.tensor_scalar(
                            out=sg[:, g * NB:(g + 1) * NB],
                            in0=rv[:, s0:s0 + BLK, g * BC:(g + 1) * BC],
                            scalar1=THR, scalar2=None,
                            op0=ALU.is_gt,
                        )

            def fc_mm(b):
                """fc (3 fp16 matmuls) of block b's spikes into bank 3 of
                the PSUM tile of block b+LAG."""
                if b < 0 or b >= fcblk:
                    return
                sg = sgs[b]
                for g in range(G):
                    nc.tensor.matmul(
                        out=chs[b + LAG][0:J, G * NB:4 * NB],
                        lhsT=fcw[:, g * J:(g + 1) * J],
                        rhs=sg[:, g * NB:(g + 1) * NB],
                        start=False, stop=(g == G - 1),
                        skip_group_check=True,
                    )

            def stage(b):
                """4 ACT copies permute PSUM tile b (g-major) into the
                t-major fp32 SBUF ctile the LIF op reads."""
                if b >= nblk or b in cts:
                    return
                ct = ctp.tile([128, BLK * 128], F32, tag="ctile")
                cts[b] = ct
                cv = ct[:].rearrange("p (t c) -> p t c", c=128)
                for g in range(4):
                    nc.scalar.copy(
                        out=cv[:, :, g * BC:(g + 1) * BC],
                        in_=chs[b][:, g * NB:(g + 1) * NB],
                    )

            def hist_dma(b):
                """mem2 of DVE-tick block b = m2 ticks [16b-SKEW, ...):
                DMA straight from the ring to DRAM (host sums)."""
                t0 = b * BLK - SKEW
                if t0 < 0:
                    return
                n = min(BLK, T - t0)
                if n <= 0:
                    return
                ring, h = rtile(b)
                rv = ring[:].rearrange("p (t c) -> p t c", c=128)
                s0 = 1 + h * BLK
                nc.sync.dma_start(
                    hist_d[:, t0 * BC:(t0 + n) * BC],
                    rv[0:J, s0:s0 + n, G * BC:128],
                )

            # prologue: two windows ahead; PSUM tiles 0-1 and ctile 0
            # ready before the loop so LIF(0) starts immediately.
            load_window(0)
            load_window(1)
            load_window(2)
            psum_bias(0)
            psum_conv(0)
            psum_bias(1)
            psum_conv(1)
            stage(0)

            for b in range(nblk):
                if (b * BLK) % WIN == 0 and b > 0:
                    load_window(b * BLK // WIN + 2)
                hist_dma(b - 1)

                # ---- critical chain: seed + the 16-steps-in-one-op ----
                ring, h = rtile(b)
                n = min(BLK, ticks - b * BLK) * 128
                if h == 0 and b > 0:
                    prev, _ = rtile(b - 1)
                    nc.vector.tensor_copy(
                        out=ring[:, 0:128],
                        in_=prev[:, TBLK * BLK * 128:TBLK * BLK * 128 + 128],
                    )
                o0 = (1 + h * BLK) * 128
                nc.vector._custom_dve(
                    lif,
                    out=ring[:, o0:o0 + n],
                    in0=ring[:, o0 - 128:o0 - 128 + n],
                    in1=cts[b][:, 0:n],
                    s0=BETA, s1=THR,
                )

                # ---- pipeline refill (emitted after the LIF op so its
                # wait thresholds never cover this block's work).  fc of
                # block b-1's spikes is emitted HERE (one block late) so a
                # PSUM tile is written over 1 period and staged the next —
                # only 2 tiles (8 banks) ever live. ----
                spikes(b)                  # GPSIMD: spikes of block b
                psum_bias(b + 2)           # PE: bias before fc accumulate
                fc_mm(b - 1)               # PE: fc -> tile (b-1)+LAG = b+2
                stage(b + 1)               # ACT: ctile for the next block
                psum_conv(b + 2)           # PE: conv for tile b+2

            # epilogue: the last block's mem2 history
            hist_dma(nblk - 1)

    nc.compile()
    return nc


def _bf16_split(a):
    import ml_dtypes
    hi = a.astype(ml_dtypes.bfloat16)
    lo = (a - hi.astype(np.float32)).astype(ml_dtypes.bfloat16)
    return hi, lo


def _host_prep(x, conv_w, conv_b, fc_w, fc_b, T):
    """Build per-core input maps (numpy only)."""
    import ml_dtypes
    ticks = T + SKEW
    nblk = -(-ticks // BLK)
    windows = -(-(nblk * BLK) // WIN)
    xt_ticks = windows * WIN

    wexp = np.zeros((KX, F), np.float32)
    for c in range(CH):
        for l in range(LO):
            wexp[l:l + 7, c * LO + l] = conv_w[c, 0, :]
        wexp[L_IN, c * LO:(c + 1) * LO] = conv_b[c]
    weh, wel = _bf16_split(wexp)
    wes = np.concatenate([weh, weh, wel], axis=0)  # K-stacked [93, F]

    if SIGN_ON_ACT:
        # sigma = sign(m-1) in {-1,0,1}: c2 = (fc_w/2) @ sigma + adjusted b
        wmat = (fc_w * 0.5).astype(np.float32)
        badd = wmat.sum(axis=1)
    else:
        # s = (m > 1) in {0,1}: plain weights and bias
        wmat = fc_w.astype(np.float32)
        badd = 0.0
    fcwt = np.zeros((128, G * J), np.float32)
    for g in range(G):
        fcwt[:, g * J:(g + 1) * J] = wmat[:, g * 128:(g + 1) * 128].T
    fcw = fcwt.astype(np.float16)
    brow = np.zeros((1, 128), np.float32)
    brow[0, :J] = fc_b + badd
    brh, brl = _bf16_split(brow)
    brs = np.concatenate([brh, brl], axis=0)       # [2, 128]

    ones = np.ones((2, BLK * BC), ml_dtypes.bfloat16)

    in_maps = []
    B = x.shape[0]
    n_cores = B // BC
    for core in range(n_cores):
        xc = x[core * BC:(core + 1) * BC]          # [BC, T, L]
        xt = np.zeros((KX, xt_ticks, BC), np.float32)
        xt[:L_IN, :T, :] = xc.transpose(2, 1, 0)
        xt[L_IN, :T, :] = 1.0
        xt = xt.reshape(KX, xt_ticks * BC)
        xth, xtl = _bf16_split(xt)
        xstk = np.concatenate([xth, xtl, xth], axis=0)  # [93, cols]
        in_maps.append({
            "xts": xstk, "wes": wes, "fcw": fcw,
            "brs": brs, "ones": ones,
        })
    return in_maps


def _install_trace_hook():
    """Wire up the axon NTFF profiling hook (absent from this image)."""
    import types

    if "antenv.axon_hooks" in sys.modules:
        return True
    try:
        if "/root/.axon_site" not in sys.path:
            sys.path.insert(0, "/root/.axon_site")
        from trn_agent_boot.trn_boot import _ntff_profile_via_ctypes

        hook = _ntff_profile_via_ctypes("/opt/axon/libaxon_pjrt.so")
        if hook is None:
            return False
        mod = types.ModuleType("antenv.axon_hooks")
        mod.get_axon_ntff_profile_hook = lambda: hook
        sys.modules["antenv.axon_hooks"] = mod
        import concourse.bass_utils as bu

        bu.upload_artifacts = lambda tmpdir: str(tmpdir)
        return True
    except Exception as e:  # profiling is optional
        print(f"trace hook install failed: {e}", file=sys.stderr)
        return False


def run_cores(x, conv_w, conv_b, fc_w, fc_b, T=None):
    """Run the Bass kernel on len(batch)/32 cores; returns [B, 35] output."""
    global LAST_RESULTS
    T = T if T is not None else x.shape[1]
    trace = TRACE and _install_trace_hook()
    nc = _build_nc(T)
    in_maps = _host_prep(x, conv_w, conv_b, fc_w, fc_b, T)
    res = run_bass_kernel_spmd(
        nc, in_maps, core_ids=list(range(len(in_maps))), trace=trace,
    )
    LAST_RESULTS = res
    outs = []
    for i in range(len(in_maps)):
        hv = np.asarray(res.results[i]["hist"], dtype=np.float32)
        m2 = hv.reshape(J, T, BC)                  # [J, t, sample]
        outs.append((m2.sum(axis=1) / np.float32(T)).T.astype(np.float32))
    return np.concatenate(outs, axis=0)


def kernel(x, conv_w, conv_b, fc_w, fc_b):
    return run_cores(
        np.asarray(x, np.float32), np.asarray(conv_w, np.float32),
        np.asarray(conv_b, np.float32), np.asarray(fc_w, np.float32),
        np.asarray(fc_b, np.float32),
    )


# revision 31
# speedup vs baseline: 5.3493x; 5.3493x over previous
"""Trainium2 Bass kernel for nn_BasicClassifier (spiking conv classifier).

Sharding: pure data parallelism — batch 256 is split 32 samples per core
across 8 NeuronCores; params are replicated (tiny).

Per-core design. The T=1000 LIF scan is sequential, so per-step cost on the
pacing engine (DVE) decides everything.  The key trick: ONE custom-DVE op
per 16-tick block computes all 16 sequential LIF steps, exploiting the
DVE's in-instruction write->read visibility at a 128-element lag (measured
on HW: reads of an address written by the same instruction 128 cycles
earlier return the new value; at <=96 cycles they race).

State is t-major (tick-slot-major) with seed slots:
  ring tile (fp32 SBUF, 2 tiles of TBLK=2 blocks each): 33 slots of
  [128 cols]; slot 0 = seed (m one tick before the tile), slot 1+k = m
  after tick k.  Within a slot: cols g*32+s, groups 0..2 = layer-1
  membrane (feature f = g*128+p at partition p), group 3 = layer-2
  membrane [35 x 32], lagged SKEW=48 ticks.

  Per block the LIF op is  m' = (m*0.9 + c) - (m > 1)  over 2048
  contiguous elements (16 slots x 128) with in0 = out SHIFTED ONE SLOT
  (128 elements = 128 DVE cycles): 16 per-tick instructions (~320ns
  each) collapse into one ~2.3us op (measured: the op runs 1 elem/cycle
  with the SBUF fp32 in1 — no read-port penalty at this shape).  A tiny
  DVE copy seeds slot 0 of the next ring tile every TBLK blocks.

Per 16-tick block (all off the tick-critical path; LAG=3 blocks of
layer-2 lag gives every pipeline stage a full block-period of slack):
  - PSUM C tile [128, 4*512] (bank-contiguous, g-major — the PE cannot
    write strided cross-bank outputs): banks 0-2 = conv1d hoisted into
    [31 -> 384] GEMMs (bf16 hi/lo K-stacked: xh@Wh + xl@Wh + xh@Wl in
    one K=93 matmul per group; ones row folds conv_b), bank 3 = fc bias
    prefill (K=2 bf16 hi/lo matmul, exact) + 3 fp16 fc matmuls from the
    spikes of block b-LAG.
  - staging: 4 ACT copies permute the PSUM tile into the t-major fp32
    SBUF ctile the LIF op consumes ([128,512] bank g -> strided
    [p][slot:128][32 @ g*32]).  Exact (fp32 -> fp32).
  - spikes: 3 GPSIMD tensor_scalar ops (strided [128,16,32] in,
    group-contiguous out): s = (m1 > 1) in {0,1} fp16; fc weights are
    plain fc_w fp16 (no sigma folding needed).
  - mem2 history: DMA of ring slot cols 96:128 to fp32 [35, 32*T] DRAM
    (host sums over time).
x is streamed in 16-tick chunk DMAs on the sync queue (chunks small
enough that the per-block hist DMAs are never head-of-line blocked for
long), prefetched ~2 windows ahead.
"""

import os
import sys

for _p in ("/opt/trn_rl_repo", "/opt/pypackages"):
    if _p not in sys.path:
        sys.path.insert(0, _p)

import numpy as np

import concourse.bacc as bacc
import concourse.mybir as mybir
import concourse.tile as tile
import concourse.dve_ops as dve_ops
from concourse.dve_spec import Spec, Src0, Src1, C0, C1, lower
from concourse.dve_uop import DveOpSpec
from concourse.bass_utils import run_bass_kernel_spmd

F32 = mybir.dt.float32
F16 = mybir.dt.float16
BF16 = mybir.dt.bfloat16
ALU = mybir.AluOpType
AF = mybir.ActivationFunctionType
AX = mybir.AxisListType

N_CORES = 8
B_FULL, T_FULL, L_IN = 256, 1000, 30
BC = B_FULL // N_CORES      # 32 samples per core
CH, LO = 16, 24
F = CH * LO                 # 384 features
G = 3                       # feature groups of 128
J = 35                      # fc outputs
KX = L_IN + 1               # conv contraction rows (30 taps + ones row)
BLK = 16                    # ticks per block (N = 16*32 = 512 = 1 PSUM bank)
LAG = 3                     # layer-2 lag in blocks
SKEW = LAG * BLK            # c2_t consumed at DVE tick t+SKEW
WIN = 160                   # ticks per x-window (chunk-DMAed per block)
TBLK = 2                    # ring tile = TBLK blocks
BETA, THR = 0.9, 1.0
SIGN_ON_ACT = True          # ACT Sign + sigma/halved-weight trick

TRACE = bool(int(os.environ.get("KERNEL_TRACE", "0")))
LAST_RESULTS = None

_LIF_OP = None


def _get_lif_op():
    """Register the fused LIF-step op in the custom-DVE table (idempotent)."""
    global _LIF_OP
    if _LIF_OP is not None:
        return _LIF_OP
    name = "LIF_STEP_ANT59"
    for op in dve_ops.OPS:
        if op.name == name:
            _LIF_OP = op
            return op
    spec = Spec(
        body=(Src0 * C0 + Src1) - (Src0 > C1),
        reference=lambda in0, in1, s0, s1, imm2: (
            (in0.astype(np.float32) * np.float32(s0)
             + in1.reshape(in0.shape))
            - (in0 > s1).astype(np.float32)
        ).astype(np.float32),
    )
    row = dve_ops._CUSTOM_DVE_ROW_BASE + len(dve_ops.OPS)
    assert row < 0x20
    dve_ops._SUB_OPCODE_FOR_NAME[name] = row
    shas = {}
    for ver in ("v3", "v4"):
        tmp = DveOpSpec(name=name, opcode=row, uops=lower(spec, ver=ver), rd1_en=True)
        shas[ver] = tmp.sha(ver)
    op = dve_ops.DveOp(name, spec, subdim=False, uops_sha=shas)
    dve_ops.OPS.append(op)
    dve_ops.CUSTOM_DVE_SPECS[name] = spec
    _LIF_OP = op
    return op


def _build_nc(T):
    """Build the per-core Bass program (SPMD: same program on every core)."""
    lif = _get_lif_op()
    ticks = T + SKEW                       # DVE ticks 0..T+SKEW-1
    nblk = -(-ticks // BLK)
    pad_ticks = nblk * BLK
    windows = -(-pad_ticks // WIN)
    xt_cols = windows * WIN * BC
    fcblk = -(-T // BLK)                   # blocks that need spikes/fc
    NB = BLK * BC                          # 512: one PSUM bank of f32
    SLOTS = TBLK * BLK + 1                 # ring slots (33), 128 cols each

    nc = bacc.Bacc("TRN2", target_bir_lowering=False)

    KS = 3 * KX                            # stacked conv K: [xh; xl; xh]
    xts_d = nc.dram_tensor("xts", [KS, xt_cols], BF16, kind="ExternalInput")
    wes_d = nc.dram_tensor("wes", [KS, F], BF16, kind="ExternalInput")
    fcw_d = nc.dram_tensor("fcw", [128, G * J], F16, kind="ExternalInput")
    brs_d = nc.dram_tensor("brs", [2, 128], BF16, kind="ExternalInput")
    ones_d = nc.dram_tensor("ones", [2, NB], BF16, kind="ExternalInput")
    hist_d = nc.dram_tensor("hist", [J, BC * T], F32, kind="ExternalOutput")

    with tile.TileContext(nc) as tc:
        with (
            tc.tile_pool(name="konst", bufs=1) as kp,
            tc.tile_pool(name="ring", bufs=1) as rp,
            tc.tile_pool(name="sig", bufs=2) as sgp,
            tc.tile_pool(name="ct", bufs=2) as ctp,
            tc.tile_pool(name="xwin", bufs=3) as xp,
            tc.tile_pool(name="cpsum", bufs=2, space="PSUM") as cp,
        ):
            # constants -> SBUF
            wes = kp.tile([KS, F], BF16, tag="wes")
            fcw = kp.tile([128, G * J], F16, tag="fcw")
            brs = kp.tile([2, 128], BF16, tag="brs")
            ones = kp.tile([2, NB], BF16, tag="ones")
            negthr = kp.tile([128, 1], F32, tag="negthr")
            nc.vector.memset(negthr[:], -THR)
            for sb, dr in ((wes, wes_d), (fcw, fcw_d),
                           (brs, brs_d), (ones, ones_d)):
                nc.sync.dma_start(sb[:], dr[:])

            # state ring: 2 tiles of TBLK blocks (t-major, 33 slots x 128)
            ringA = rp.tile([128, SLOTS * 128], F32, tag="ringA")
            ringB = rp.tile([128, SLOTS * 128], F32, tag="ringB")
            nc.vector.memset(ringA[:], 0.0)
            nc.vector.memset(ringB[:], 0.0)
            rings = (ringA, ringB)

            xts = {}      # window idx -> xt sbuf tile
            chs = {}      # block idx -> PSUM C tile [128, 4*512] g-major
            cts = {}      # block idx -> SBUF C tile [128, 2048] t-major
            sgs = {}      # block idx -> spike tile

            def rtile(b):
                """(ring tile, block-within-tile h) for block b."""
                return rings[(b // TBLK) % 2], b % TBLK

            def load_window(w):
                """x window as per-block chunk DMAs on the sync queue;
                small chunks never head-of-line-block a hist DMA for more
                than ~1.5us on the DMA engine."""
                if w >= windows or w in xts:
                    return
                ts = xp.tile([KS, WIN * BC], BF16, tag="xws")
                for c in range(WIN // BLK):
                    nc.sync.dma_start(
                        ts[:, c * NB:(c + 1) * NB],
                        xts_d[:, w * WIN * BC + c * NB:
                                 w * WIN * BC + (c + 1) * NB],
                    )
                xts[w] = ts

            def psum_bias(b):
                """Allocate block b's PSUM C tile; prime bank 3 with the fc
                bias (zeros when block b-LAG has no spikes)."""
                if b >= nblk or b in chs:
                    return
                ch = cp.tile([128, 4 * NB], F32, tag="ch")
                chs[b] = ch
                if b >= LAG and b - LAG < fcblk:
                    nc.tensor.matmul(
                        out=ch[:, G * NB:4 * NB],
                        lhsT=brs[:, :], rhs=ones[:, :],
                        start=True, stop=False,
                        skip_group_check=True,
                    )
                else:
                    nc.vector.memset(ch[:, G * NB:4 * NB], 0.0)

            def psum_conv(b):
                """conv-fill banks 0-2 of block b's PSUM C tile."""
                if b >= nblk:
                    return
                ch = chs[b]
                w = (b * BLK) // WIN
                base = (b * BLK - w * WIN) * BC
                for g in range(G):
                    nc.tensor.matmul(
                        out=ch[:, g * NB:(g + 1) * NB],
                        lhsT=wes[:, g * 128:(g + 1) * 128],
                        rhs=xts[w][:, base:base + NB],
                        start=True, stop=True,
                    )

            def spikes(b):
                """sigma = Sign(m1 - 1) in {-1,0,1} fp16 — 3 ACT ops
                (strided in, group-contiguous out).  (GPSIMD tensor_scalar
                measured ~19 cycles/elem AND floods the SBUF port it
                shares with the DVE — keep GPSIMD compute-free.)"""
                if b < 0 or b >= fcblk:
                    return
                ring, h = rtile(b)
                rv = ring[:].rearrange("p (t c) -> p t c", c=128)
                s0 = 1 + h * BLK
                sg = sgp.tile([128, G * NB], F16, tag="sg")
                sgs[b] = sg
                for g in range(G):
                    nc.scalar.activation(
                        out=sg[:, g * NB:(g + 1) * NB],
                        in_=rv[:, s0:s0 + BLK, g * BC:(g + 1) * BC],
                        func=AF.Sign, bias=negthr[:],
                    )

            def fc_mm(b):
                """fc (3 fp16 matmuls) of block b's spikes into bank 3 of
                the PSUM tile of block b+LAG."""
                if b < 0 or b >= fcblk:
                    return
                sg = sgs[b]
                for g in range(G):
                    nc.tensor.matmul(
                        out=chs[b + LAG][0:J, G * NB:4 * NB],
                        lhsT=fcw[:, g * J:(g + 1) * J],
                        rhs=sg[:, g * NB:(g + 1) * NB],
                        start=False, stop=(g == G - 1),
                        skip_group_check=True,
                    )

            def stage(b, part):
                """Permute PSUM tile b (g-major) into the t-major fp32
                SBUF ctile the LIF op reads — one copy per group, split
                across engines to balance the block budget: part 0 =
                groups 0-1 on ACT, part 1 = groups 2-3 on the DVE (the
                DVE has headroom next to its one block op)."""
                if b >= nblk:
                    return
                if b not in cts:
                    cts[b] = ctp.tile([128, BLK * 128], F32, tag="ctile",
                                      name="ctile")
                cv = cts[b][:].rearrange("p (t c) -> p t c", c=128)
                for g in (0, 1) if part == 0 else (2, 3):
                    if part == 0:
                        nc.scalar.copy(
                            out=cv[:, :, g * BC:(g + 1) * BC],
                            in_=chs[b][:, g * NB:(g + 1) * NB],
                        )
                    else:
                        nc.vector.tensor_copy(
                            out=cv[:, :, g * BC:(g + 1) * BC],
                            in_=chs[b][:, g * NB:(g + 1) * NB],
                        )

            def hist_dma(b):
                """mem2 of DVE-tick block b = m2 ticks [16b-SKEW, ...):
                DMA straight from the ring to DRAM (host sums)."""
                t0 = b * BLK - SKEW
                if t0 < 0:
                    return
                n = min(BLK, T - t0)
                if n <= 0:
                    return
                ring, h = rtile(b)
                rv = ring[:].rearrange("p (t c) -> p t c", c=128)
                s0 = 1 + h * BLK
                nc.sync.dma_start(
                    hist_d[:, t0 * BC:(t0 + n) * BC],
                    rv[0:J, s0:s0 + n, G * BC:128],
                )

            # prologue: two windows ahead; PSUM tiles 0-1 and ctile 0
            # ready before the loop so LIF(0) starts immediately.
            load_window(0)
            load_window(1)
            load_window(2)
            psum_bias(0)
            psum_conv(0)
            psum_bias(1)
            psum_conv(1)
            stage(0, 0)
            stage(0, 1)

            for b in range(nblk):
                if (b * BLK) % WIN == 0 and b > 0:
                    load_window(b * BLK // WIN + 2)
                hist_dma(b - 1)

                # ---- critical chain: seed + the 16-steps-in-one-op ----
                ring, h = rtile(b)
                n = min(BLK, ticks - b * BLK) * 128
                if h == 0 and b > 0:
                    prev, _ = rtile(b - 1)
                    nc.vector.tensor_copy(
                        out=ring[:, 0:128],
                        in_=prev[:, TBLK * BLK * 128:TBLK * BLK * 128 + 128],
                    )
                o0 = (1 + h * BLK) * 128
                nc.vector._custom_dve(
                    lif,
                    out=ring[:, o0:o0 + n],
                    in0=ring[:, o0 - 128:o0 - 128 + n],
                    in1=cts[b][:, 0:n],
                    s0=BETA, s1=THR,
                )

                # ---- pipeline refill (emitted after the LIF op so its
                # wait thresholds never cover this block's work).  fc of
                # block b-1's spikes is emitted HERE (one block late) so a
                # PSUM tile is written over 1 period and staged the next —
                # only 2 tiles (8 banks) ever live. ----
                stage(b + 1, 1)            # DVE: its share of the ctile
                spikes(b)                  # ACT: spikes of block b
                psum_bias(b + 2)           # PE: bias before fc accumulate
                fc_mm(b - 1)               # PE: fc -> tile (b-1)+LAG = b+2
                stage(b + 1, 0)            # ACT: its share of the ctile
                psum_conv(b + 2)           # PE: conv for tile b+2

            # epilogue: the last block's mem2 history
            hist_dma(nblk - 1)

    nc.compile()
    return nc


def _bf16_split(a):
    import ml_dtypes
    hi = a.astype(ml_dtypes.bfloat16)
    lo = (a - hi.astype(np.float32)).astype(ml_dtypes.bfloat16)
    return hi, lo


def _host_prep(x, conv_w, conv_b, fc_w, fc_b, T):
    """Build per-core input maps (numpy only)."""
    import ml_dtypes
    ticks = T + SKEW
    nblk = -(-ticks // BLK)
    windows = -(-(nblk * BLK) // WIN)
    xt_ticks = windows * WIN

    wexp = np.zeros((KX, F), np.float32)
    for c in range(CH):
        for l in range(LO):
            wexp[l:l + 7, c * LO + l] = conv_w[c, 0, :]
        wexp[L_IN, c * LO:(c + 1) * LO] = conv_b[c]
    weh, wel = _bf16_split(wexp)
    wes = np.concatenate([weh, weh, wel], axis=0)  # K-stacked [93, F]

    if SIGN_ON_ACT:
        # sigma = sign(m-1) in {-1,0,1}: c2 = (fc_w/2) @ sigma + adjusted b
        wmat = (fc_w * 0.5).astype(np.float32)
        badd = wmat.sum(axis=1)
    else:
        # s = (m > 1) in {0,1}: plain weights and bias
        wmat = fc_w.astype(np.float32)
        badd = 0.0
    fcwt = np.zeros((128, G * J), np.float32)
    for g in range(G):
        fcwt[:, g * J:(g + 1) * J] = wmat[:, g * 128:(g + 1) * 128].T
    fcw = fcwt.astype(np.float16)
    brow = np.zeros((1, 128), np.float32)
    brow[0, :J] = fc_b + badd
    brh, brl = _bf16_split(brow)
    brs = np.concatenate([brh, brl], axis=0)       # [2, 128]

    ones = np.ones((2, BLK * BC), ml_dtypes.bfloat16)

    in_maps = []
    B = x.shape[0]
    n_cores = B // BC
    for core in range(n_cores):
        xc = x[core * BC:(core + 1) * BC]          # [BC, T, L]
        xt = np.zeros((KX, xt_ticks, BC), np.float32)
        xt[:L_IN, :T, :] = xc.transpose(2, 1, 0)
        xt[L_IN, :T, :] = 1.0
        xt = xt.reshape(KX, xt_ticks * BC)
        xth, xtl = _bf16_split(xt)
        xstk = np.concatenate([xth, xtl, xth], axis=0)  # [93, cols]
        in_maps.append({
            "xts": xstk, "wes": wes, "fcw": fcw,
            "brs": brs, "ones": ones,
        })
    return in_maps


def _install_trace_hook():
    """Wire up the axon NTFF profiling hook (absent from this image)."""
    import types

    if "antenv.axon_hooks" in sys.modules:
        return True
    try:
        if "/root/.axon_site" not in sys.path:
            sys.path.insert(0, "/root/.axon_site")
        from trn_agent_boot.trn_boot import _ntff_profile_via_ctypes

        hook = _ntff_profile_via_ctypes("/opt/axon/libaxon_pjrt.so")
        if hook is None:
            return False
        mod = types.ModuleType("antenv.axon_hooks")
        mod.get_axon_ntff_profile_hook = lambda: hook
        sys.modules["antenv.axon_hooks"] = mod
        import concourse.bass_utils as bu

        bu.upload_artifacts = lambda tmpdir: str(tmpdir)
        return True
    except Exception as e:  # profiling is optional
        print(f"trace hook install failed: {e}", file=sys.stderr)
        return False


def run_cores(x, conv_w, conv_b, fc_w, fc_b, T=None):
    """Run the Bass kernel on len(batch)/32 cores; returns [B, 35] output."""
    global LAST_RESULTS
    T = T if T is not None else x.shape[1]
    trace = TRACE and _install_trace_hook()
    nc = _build_nc(T)
    in_maps = _host_prep(x, conv_w, conv_b, fc_w, fc_b, T)
    res = run_bass_kernel_spmd(
        nc, in_maps, core_ids=list(range(len(in_maps))), trace=trace,
    )
    LAST_RESULTS = res
    outs = []
    for i in range(len(in_maps)):
        hv = np.asarray(res.results[i]["hist"], dtype=np.float32)
        m2 = hv.reshape(J, T, BC)                  # [J, t, sample]
        outs.append((m2.sum(axis=1) / np.float32(T)).T.astype(np.float32))
    return np.concatenate(outs, axis=0)


def kernel(x, conv_w, conv_b, fc_w, fc_b):
    return run_cores(
        np.asarray(x, np.float32), np.asarray(conv_w, np.float32),
        np.asarray(conv_b, np.float32), np.asarray(fc_w, np.float32),
        np.asarray(fc_b, np.float32),
    )
